# revision 6
# baseline (speedup 1.0000x reference)
"""Trainium2 Bass kernel for nn_CrossAttention (B=8, N=M=2048, C=512, H=4).

Sharding: data-parallel over batch - one batch element per NeuronCore (8 cores).

Per-core dataflow (v2: fp8 DoubleRow for the attention*value and denominator
matmuls; everything else fp16 operands with fp32 PSUM accumulation):

  1. F1^T, F2^T via PE transposes. F1/F2 are cast fp32->fp16 by the DGE
     casting DMA on load, so transposes run at 1 cycle/row.
  2. q^T  = (F1 @ W + b)^T : lhsT=W-chunk, rhs=F1^T  (d-major fp16)
     kv^T = (F2 @ W + b)^T : lhsT=W-chunk, rhs=F2^T  (d-major fp16)
     kvn2 = kv in natural (m-major) layout, fp8e4, stored as mb-PAIRS
     [P, jj, 2, C] so it can be the DoubleRow stationary operand.
  3. per (n-stripe of 512, head), software-pipelined one iteration deep:
       scores^T[m,n] = kv_h^T.T @ q_h^T          (fp16 PE, PSUM f32)
       E = exp(SCALE*s)                          (ACT, PSUM->SBUF, fp8e4;
                                                  |SCALE*s| <= ~2, no max-sub)
       pv^T[d,n] = sum_mb kvn2-pairs.T @ E-pairs (fp8 DoubleRow, 2x rate)
       dn[1,n]   = sum_mb ones-pairs.T @ E-pairs (fp8 DoubleRow ones-matmul)
       recip on [1,n] (DVE), partition-broadcast (GPSIMD), xT = pv*recip (DVE)
     The pv/dn matmuls for iteration t-1 are interleaved between the score
     matmuls of iteration t: this keeps PSUM at 8 banks (sc 2x2 + pv 1 +
     dn 1 + proj/out 2) and gives the PE ready work while ACT runs exp.
  4. out[n,c] = xT-blocks.T @ W_proj (fp16); bias added by DVE during the
     PSUM evacuation (pre-broadcast bias tile), DMA out. Emitted per stripe
     as soon as all 4 heads of that stripe are done, overlapping attention.
"""
import sys

for _p in ("/opt/trn_rl_repo", "/root/.axon_site/_ro/trn_rl_repo"):
    if _p not in sys.path:
        sys.path.insert(0, _p)

import numpy as np
import concourse.bass as bass
import concourse.bacc as bacc
import concourse.tile as tile
from concourse import mybir
from concourse.bass_utils import run_bass_kernel_spmd

F32 = mybir.dt.float32
F16 = mybir.dt.float16
FP8 = mybir.dt.float8e4
EXP = mybir.ActivationFunctionType.Exp
DR = mybir.MatmulPerfMode.DoubleRow

B, N, M, C = 8, 2048, 2048, 512
H, D = 4, 128
SCALE = 1.0 / np.sqrt(C)
P = 128
NB = N // P        # 16 n-blocks
MB = M // P        # 16 m-blocks
KC = C // P        # 4 contraction chunks (= heads since D=128)
NS = 4             # n-stripes of 512
SW = N // NS       # stripe width 512
JJ = MB // 2       # 8 mb-pairs


def build_nc():
    nc = bacc.Bacc(None, target_bir_lowering=False)
    dF1 = nc.dram_tensor("F1", [N, C], F32, kind="ExternalInput")
    dF2 = nc.dram_tensor("F2", [M, C], F32, kind="ExternalInput")
    dW = nc.dram_tensor("Wqkv", [C, C], F32, kind="ExternalInput")
    dBq = nc.dram_tensor("bqkv", [1, C], F32, kind="ExternalInput")
    dWp = nc.dram_tensor("Wproj", [C, C], F32, kind="ExternalInput")
    dBp = nc.dram_tensor("bproj", [1, C], F32, kind="ExternalInput")
    dOut = nc.dram_tensor("OUT", [N, C], F32, kind="ExternalOutput")

    d_ident16 = nc.inline_tensor(np.eye(P, dtype=np.float16), name="identity16")
    d_ones2 = nc.inline_tensor(np.ones((P, 2, 32), np.float16), name="ones2")

    with tile.TileContext(nc) as tc:
        with (
            tc.tile_pool(name="const", bufs=1) as const,
            tc.tile_pool(name="persist", bufs=1) as persist,
        ):
            # ---- constants ----
            ident16 = const.tile([P, P], F16)
            nc.sync.dma_start(ident16, d_ident16[:])
            ones16 = const.tile([P, 2, 32], F16)
            nc.sync.dma_start(ones16, d_ones2[:])
            ones8 = const.tile([P, 2, 32], FP8)
            with nc.allow_low_precision(reason="ones are exact in fp8"):
                nc.vector.tensor_copy(ones8, ones16)
            bq_col = const.tile([P, KC], F32)
            nc.sync.dma_start(bq_col, dBq[0, :].rearrange("(a b) -> b a", b=P))
            bp_row = const.tile([1, C], F32)
            nc.sync.dma_start(bp_row, dBp[:])
            bpb = const.tile([P, C], F32)
            nc.gpsimd.partition_broadcast(bpb, bp_row)

            # weights fp32 via sync DMA, cast to fp16 on DVE (cheap, early)
            W16, Wp16 = [], []
            with tc.tile_pool(name="wtmp", bufs=2) as wtmp:
                for kc in range(KC):
                    for dsrc, lst, nm in ((dW, W16, "w"), (dWp, Wp16, "wp")):
                        w32 = wtmp.tile([P, C], F32, tag="w32", name="w32")
                        nc.sync.dma_start(w32, dsrc[kc * P:(kc + 1) * P, :])
                        w16 = persist.tile([P, C], F16, name=f"{nm}16_{kc}")
                        with nc.allow_low_precision(reason="fp16 weights"):
                            nc.vector.tensor_copy(w16, w32)
                        lst.append(w16)

            # ---- persistent activations ----
            FT = {
                t: [persist.tile([P, N], F16, name=f"{t}T{i}") for i in range(KC)]
                for t in ("f2", "f1")
            }
            qT = [persist.tile([P, N], F16, name=f"qT{i}") for i in range(KC)]
            kvT = [persist.tile([P, N], F16, name=f"kvT{i}") for i in range(KC)]
            kvn2 = persist.tile([P, JJ, 2, C], FP8, name="kvn2")
            xT = [persist.tile([P, N], F16, name=f"xT{i}") for i in range(KC)]

            with tc.tile_pool(name="pj", bufs=2, space="PSUM") as pjps:

                def emit_qkvT(dst, src):
                    # dst^T[c_out, n] = sum_kc W[kc,c_out].T @ src^T[kc, n] + b
                    for co in range(KC):
                        for g in range(NS):
                            pjt = pjps.tile([P, SW], F32, tag="pj", name="pjt")
                            for kc in range(KC):
                                nc.tensor.matmul(
                                    pjt,
                                    W16[kc][:, co * P:(co + 1) * P],
                                    src[kc][:, g * SW:(g + 1) * SW],
                                    start=(kc == 0),
                                    stop=(kc == KC - 1),
                                )
                            nc.vector.tensor_scalar_add(
                                dst[co][:, g * SW:(g + 1) * SW],
                                pjt,
                                bq_col[:, co:co + 1],
                            )

                # ---- phase 1+2 (F2 first so attention can start early) ----
                with (
                    tc.tile_pool(name="fin", bufs=6) as fpool,
                    tc.tile_pool(name="trps", bufs=4, space="PSUM") as trps,
                    tc.tile_pool(name="kvnps", bufs=2, space="PSUM") as kvnps,
                ):
                    def emit_ft(tag, dsrc):
                        # casting DMA (f32->f16) then PE transpose per block
                        for g in range(NS):
                            tp = [
                                trps.tile([P, SW], F16, tag="trp",
                                          name=f"trp_{tag}_{g}_{k}")
                                for k in range(KC)
                            ]
                            for i in range(4):
                                nb = 4 * g + i
                                fin = fpool.tile([P, C], F16, tag="fin",
                                                 name="fin")
                                nc.gpsimd.dma_start(
                                    fin, dsrc[nb * P:(nb + 1) * P, :]
                                )
                                for kc in range(KC):
                                    nc.tensor.transpose(
                                        tp[kc][:, i * P:(i + 1) * P],
                                        fin[:, kc * P:(kc + 1) * P],
                                        ident16,
                                    )
                            for kc in range(KC):
                                nc.vector.tensor_copy(
                                    FT[tag][kc][:, g * SW:(g + 1) * SW],
                                    tp[kc],
                                )

                    emit_ft("f2", dF2)
                    emit_qkvT(kvT, FT["f2"])
                    # kvn2[p, jj, j, hh*128+d] = kv[(2jj+j)*128+p, hh*128+d]
                    # (natural m-major kv, stored as DoubleRow mb-pairs, fp8)
                    for hh in range(H):
                        for half in range(2):
                            kt = kvnps.tile([P, 8, P], F16, tag="kvnt",
                                            name="kt")
                            for u in range(8):
                                mb = 8 * half + u
                                nc.tensor.transpose(
                                    kt[:, u, :],
                                    kvT[hh][:, mb * P:(mb + 1) * P],
                                    ident16,
                                )
                            for v in range(4):
                                jj = 4 * half + v
                                with nc.allow_low_precision(
                                    reason="fp8 kv for DoubleRow pv"
                                ):
                                    nc.vector.tensor_copy(
                                        kvn2[:, jj, :, hh * P:(hh + 1) * P],
                                        kt[:, 2 * v:2 * v + 2, :],
                                    )
                    emit_ft("f1", dF1)
                    emit_qkvT(qT, FT["f1"])

                # ---- phase 3+4: attention, software-pipelined 1 deep ----
                with (
                    tc.tile_pool(name="scps", bufs=2, space="PSUM") as scps,
                    tc.tile_pool(name="pvps", bufs=1, space="PSUM") as pvps,
                    tc.tile_pool(name="dnps", bufs=1, space="PSUM") as dnps,
                    tc.tile_pool(name="epool", bufs=2) as epool,
                    tc.tile_pool(name="ep", bufs=2) as ep,
                    tc.tile_pool(name="osb", bufs=3) as osb,
                ):
                    def emit_pv_dn(st, jp):
                        # DoubleRow: contract mb-pair 2jp,2jp+1 (256 rows)
                        if jp == 0:
                            st["pvp"] = pvps.tile([P, SW], F32, tag="pv",
                                                  name="pvp")
                            st["dnp"] = dnps.tile([32, SW], F32, tag="dn",
                                                  name="dnp")
                        E, h = st["E"], st["h"]
                        nc.tensor.matmul(
                            st["pvp"],
                            kvn2[:, jp, :, h * P:(h + 1) * P],
                            E[:, 2 * jp:2 * jp + 2, :],
                            start=(jp == 0),
                            stop=(jp == JJ - 1),
                            perf_mode=DR,
                        )
                        nc.tensor.matmul(
                            st["dnp"],
                            ones8,
                            E[:, 2 * jp:2 * jp + 2, :],
                            start=(jp == 0),
                            stop=(jp == JJ - 1),
                            perf_mode=DR,
                        )

                    def emit_epilogue(st):
                        h, s = st["h"], st["s"]
                        rec = ep.tile([1, SW], F32, tag="rec", name="rec")
                        nc.vector.reciprocal(rec, st["dnp"][0:1, :])
                        dnb = ep.tile([P, SW], F32, tag="dnb", name="dnb")
                        nc.gpsimd.partition_broadcast(dnb, rec)
                        with nc.allow_low_precision(
                            reason="x values O(0.1); fp16 keeps 5e-4 rel"
                        ):
                            nc.vector.tensor_mul(
                                xT[h][:, s * SW:(s + 1) * SW], st["pvp"], dnb
                            )

                    def emit_phase4(s):
                        for nb in range(4 * s, 4 * s + 4):
                            pr = pjps.tile([P, C], F32, tag="pj", name="pr")
                            for kc in range(KC):
                                nc.tensor.matmul(
                                    pr,
                                    xT[kc][:, nb * P:(nb + 1) * P],
                                    Wp16[kc],
                                    start=(kc == 0),
                                    stop=(kc == KC - 1),
                                )
                            ot = osb.tile([P, C], F32, tag="ot", name="ot")
                            nc.vector.tensor_add(ot, pr, bpb)
                            nc.sync.dma_start(dOut[nb * P:(nb + 1) * P, :], ot)

                    prev = None
                    for t in range(NS * H):
                        s, h = divmod(t, H)
                        E = epool.tile([P, MB, SW], FP8, tag="E", name="E")
                        cur = {"E": E, "h": h, "s": s}
                        for jp in range(JJ):
                            sc = scps.tile([P, 2, SW], F32, tag="sc",
                                           name="sc")
                            for i in range(2):
                                mb = 2 * jp + i
                                nc.tensor.matmul(
                                    sc[:, i, :],
                                    kvT[h][:, mb * P:(mb + 1) * P],
                                    qT[h][:, s * SW:(s + 1) * SW],
                                    start=True,
                                    stop=True,
                                )
                            with nc.allow_low_precision(
                                reason="fp8 attention weights; rel err ~1e-2"
                            ):
                                nc.scalar.activation(
                                    E[:, 2 * jp:2 * jp + 2, :].rearrange(
                                        "p a b -> p (a b)"
                                    ),
                                    sc.rearrange("p a b -> p (a b)"),
                                    EXP,
                                    scale=float(SCALE),
                                )
                            if prev is not None:
                                emit_pv_dn(prev, jp)
                        if prev is not None:
                            emit_epilogue(prev)
                            if prev["h"] == H - 1:
                                emit_phase4(prev["s"])
                        prev = cur
                    for jp in range(JJ):
                        emit_pv_dn(prev, jp)
                    emit_epilogue(prev)
                    emit_phase4(NS - 1)

    nc.compile()
    return nc


_NC = None


def _get_nc():
    global _NC
    if _NC is None:
        _NC = build_nc()
    return _NC


def kernel(F1, F2, W_qkv, b_qkv, W_proj, b_proj, _trace=False):
    F1 = np.ascontiguousarray(np.asarray(F1, dtype=np.float32))
    F2 = np.ascontiguousarray(np.asarray(F2, dtype=np.float32))
    W = np.ascontiguousarray(np.asarray(W_qkv, dtype=np.float32))
    bq = np.ascontiguousarray(np.asarray(b_qkv, dtype=np.float32)).reshape(1, C)
    Wpj = np.ascontiguousarray(np.asarray(W_proj, dtype=np.float32))
    bp = np.ascontiguousarray(np.asarray(b_proj, dtype=np.float32)).reshape(1, C)

    nc = _get_nc()
    in_maps = [
        {"F1": F1[b], "F2": F2[b], "Wqkv": W, "bqkv": bq, "Wproj": Wpj, "bproj": bp}
        for b in range(B)
    ]
    res = run_bass_kernel_spmd(
        nc, in_maps, core_ids=list(range(B)), trace=_trace
    )
    out = np.stack([res.results[b]["OUT"] for b in range(B)], axis=0)
    if _trace:
        return out, res
    return out


# revision 7
# speedup vs baseline: 1.3427x; 1.3427x over previous
"""Trainium2 Bass kernel for nn_CrossAttention (B=8, N=M=2048, C=512, H=4).

Sharding: data-parallel over batch - one batch element per NeuronCore (8 cores).

Per-core dataflow (v3). The scalar engine's exp over all N*M*H scores
(16.8M elems ~ 140us at 128 lanes * 1.2GHz) is the hard floor, so the
schedule is built to keep ACT 100% busy from ~15us on:

  1. F1/F2 cast fp32->fp16 by the DGE casting DMA; PE transposes to
     F^T. F2 first; kvT chunks are interleaved with F2^T per column
     group so the q/kv projection fills the DMA wait.
  2. qT/kvT = (F @ W + b)^T fp16 (d-major; bias fused into DVE evac).
     kvn2 = kv m-major mb-PAIRS [P, jj, 2, C] fp8e4 (DoubleRow
     stationary), from PE transposes of kvT16.
     Only head 0's kvT/kvn/qT are emitted before attention starts; the
     rest weave between the first attention iterations (the PE has
     ~4us/iter of slack while ACT runs exp), using the shared 2-bank
     "pj" PSUM ring. qT chunks are emitted just-in-time per (h, s).
  3. per (n-stripe of 512, head), pipelined one iteration deep:
       scores^T[m,n] (fp16 PE) -> exp (ACT, PSUM->SBUF fp8e4, no
       max-sub: |SCALE*s| <= ~2) -> pv^T[d,n] and dn[n] as fp8
       DoubleRow matmuls (2 k-tiles of 128 per pass, 0.5 cyc/col).
     pv/dn for iteration t-1 interleave between the score matmuls of
     iteration t so PSUM fits 8 banks (sc 2x2 + pv 1 + dn 1 + pj 2).
     Epilogue: DVE copy evacuates pv to SBUF (frees the PSUM bank
     without waiting on the reciprocal), reciprocal_approx_fast on the
     [1,n] denominator row, GPSIMD partition-broadcast, DVE multiply.
  4. out[n,c] = xT.T @ W_proj (fp16) per stripe as soon as its 4 heads
     finish; bias via DVE add of a pre-broadcast tile; DMA out.
"""
import sys

for _p in ("/opt/trn_rl_repo", "/root/.axon_site/_ro/trn_rl_repo"):
    if _p not in sys.path:
        sys.path.insert(0, _p)

import numpy as np
import concourse.bass as bass
import concourse.bacc as bacc
import concourse.tile as tile
from concourse import mybir
from concourse.bass_utils import run_bass_kernel_spmd

F32 = mybir.dt.float32
F16 = mybir.dt.float16
FP8 = mybir.dt.float8e4
EXP = mybir.ActivationFunctionType.Exp
DR = mybir.MatmulPerfMode.DoubleRow

B, N, M, C = 8, 2048, 2048, 512
H, D = 4, 128
SCALE = 1.0 / np.sqrt(C)
P = 128
NB = N // P        # 16 n-blocks
MB = M // P        # 16 m-blocks
KC = C // P        # 4 contraction chunks (= heads since D=128)
NS = 4             # n-stripes of 512
SW = N // NS       # stripe width 512
JJ = MB // 2       # 8 mb-pairs


def build_nc():
    nc = bacc.Bacc(None, target_bir_lowering=False)
    dF1 = nc.dram_tensor("F1", [N, C], F32, kind="ExternalInput")
    dF2 = nc.dram_tensor("F2", [M, C], F32, kind="ExternalInput")
    dW = nc.dram_tensor("Wqkv", [C, C], F32, kind="ExternalInput")
    dBq = nc.dram_tensor("bqkv", [1, C], F32, kind="ExternalInput")
    dWp = nc.dram_tensor("Wproj", [C, C], F32, kind="ExternalInput")
    dBp = nc.dram_tensor("bproj", [1, C], F32, kind="ExternalInput")
    dOut = nc.dram_tensor("OUT", [N, C], F32, kind="ExternalOutput")

    d_ident16 = nc.inline_tensor(np.eye(P, dtype=np.float16), name="identity16")
    d_ones2 = nc.inline_tensor(np.ones((P, 2, 32), np.float16), name="ones2")

    with tile.TileContext(nc) as tc:
        with (
            tc.tile_pool(name="const", bufs=1) as const,
            tc.tile_pool(name="persist", bufs=1) as persist,
        ):
            # ---- constants ----
            ident16 = const.tile([P, P], F16)
            nc.sync.dma_start(ident16, d_ident16[:])
            ones16 = const.tile([P, 2, 32], F16)
            nc.sync.dma_start(ones16, d_ones2[:])
            ones8 = const.tile([P, 2, 32], FP8)
            with nc.allow_low_precision(reason="ones are exact in fp8"):
                nc.vector.tensor_copy(ones8, ones16)
            bq_col = const.tile([P, KC], F32)
            nc.sync.dma_start(bq_col, dBq[0, :].rearrange("(a b) -> b a", b=P))
            bp_row = const.tile([1, C], F32)
            nc.sync.dma_start(bp_row, dBp[:])
            bpb = const.tile([P, C], F32)
            nc.gpsimd.partition_broadcast(bpb, bp_row)

            # weights fp32 via sync DMA, cast to fp16 on DVE (cheap, early)
            W16, Wp16 = [], []
            with tc.tile_pool(name="wtmp", bufs=2) as wtmp:
                for kc in range(KC):
                    for dsrc, lst, nm in ((dW, W16, "w"), (dWp, Wp16, "wp")):
                        w32 = wtmp.tile([P, C], F32, tag="w32", name="w32")
                        nc.sync.dma_start(w32, dsrc[kc * P:(kc + 1) * P, :])
                        w16 = persist.tile([P, C], F16, name=f"{nm}16_{kc}")
                        with nc.allow_low_precision(reason="fp16 weights"):
                            nc.vector.tensor_copy(w16, w32)
                        lst.append(w16)

            # ---- persistent activations ----
            FT = {
                t: [persist.tile([P, N], F16, name=f"{t}T{i}") for i in range(KC)]
                for t in ("f2", "f1")
            }
            qT = [persist.tile([P, N], F16, name=f"qT{i}") for i in range(KC)]
            kvT = [persist.tile([P, N], F16, name=f"kvT{i}") for i in range(KC)]
            kvn2 = persist.tile([P, JJ, 2, C], FP8, name="kvn2")
            xT = [persist.tile([P, N], F16, name=f"xT{i}") for i in range(KC)]

            # shared 2-bank PSUM ring: projections, kvn transposes, phase 4
            with tc.tile_pool(name="pj", bufs=2, space="PSUM") as pjps:

                def proj_chunk(dst, src, co, g):
                    # dst^T[co-block, g-cols] = sum_kc W[kc,co].T @ src^T + b
                    pjt = pjps.tile([P, SW], F32, tag="pj", name="pjt")
                    for kc in range(KC):
                        nc.tensor.matmul(
                            pjt,
                            W16[kc][:, co * P:(co + 1) * P],
                            src[kc][:, g * SW:(g + 1) * SW],
                            start=(kc == 0),
                            stop=(kc == KC - 1),
                        )
                    nc.vector.tensor_scalar_add(
                        dst[co][:, g * SW:(g + 1) * SW],
                        pjt,
                        bq_col[:, co:co + 1],
                    )

                def kvn_block(hh):
                    # kvn2[p,jj,j,hh*128+d] = kv[(2jj+j)*128+p, hh*128+d]
                    for half in range(2):
                        kt = pjps.tile([P, 8, P], F16, tag="pj", name="kt")
                        for u in range(8):
                            mb = 8 * half + u
                            nc.tensor.transpose(
                                kt[:, u, :],
                                kvT[hh][:, mb * P:(mb + 1) * P],
                                ident16,
                            )
                        for v in range(4):
                            jj = 4 * half + v
                            with nc.allow_low_precision(
                                reason="fp8 kv for DoubleRow pv"
                            ):
                                nc.vector.tensor_copy(
                                    kvn2[:, jj, :, hh * P:(hh + 1) * P],
                                    kt[:, 2 * v:2 * v + 2, :],
                                )

                # ---- phase 1+2 head: F2^T woven with kvT head 0 ----
                with (
                    tc.tile_pool(name="fin", bufs=6) as fpool,
                    tc.tile_pool(name="trps", bufs=2, space="PSUM") as trps,
                ):
                    def ft_group(tag, dsrc, g):
                        tp = trps.tile([P, KC, SW], F16, tag="trp", name="tp")
                        for i in range(4):
                            nb = 4 * g + i
                            fin = fpool.tile([P, C], F16, tag="fin", name="fin")
                            nc.gpsimd.dma_start(
                                fin, dsrc[nb * P:(nb + 1) * P, :]
                            )
                            for kc in range(KC):
                                nc.tensor.transpose(
                                    tp[:, kc, i * P:(i + 1) * P],
                                    fin[:, kc * P:(kc + 1) * P],
                                    ident16,
                                )
                        for kc in range(KC):
                            nc.vector.tensor_copy(
                                FT[tag][kc][:, g * SW:(g + 1) * SW],
                                tp[:, kc, :],
                            )

                    for g in range(NS):
                        ft_group("f2", dF2, g)
                        proj_chunk(kvT, FT["f2"], 0, g)
                    kvn_block(0)
                    for g in range(NS):
                        ft_group("f1", dF1, g)
                    proj_chunk(qT, FT["f1"], 0, 0)

                # work woven into the exp-paced gaps of early attention:
                # after iter t (t=0,1,2) emit head t+1's kvT + kvn
                filler = [
                    [("kvT", co, g) for g in range(NS)] + [("kvn", co)]
                    for co in (1, 2, 3)
                ]
                qT_done = {(0, 0)}

                # ---- phase 3+4: attention, software-pipelined 1 deep ----
                with (
                    tc.tile_pool(name="scps", bufs=2, space="PSUM") as scps,
                    tc.tile_pool(name="pvps", bufs=1, space="PSUM") as pvps,
                    tc.tile_pool(name="dnps", bufs=1, space="PSUM") as dnps,
                    tc.tile_pool(name="epool", bufs=2) as epool,
                    tc.tile_pool(name="ep", bufs=2) as ep,
                    tc.tile_pool(name="osb", bufs=3) as osb,
                ):
                    def emit_pv_dn(st, jp):
                        # DoubleRow: contract mb-pair 2jp,2jp+1 (256 rows)
                        if jp == 0:
                            st["pvp"] = pvps.tile([P, SW], F32, tag="pv",
                                                  name="pvp")
                            st["dnp"] = dnps.tile([32, SW], F32, tag="dn",
                                                  name="dnp")
                        E, h = st["E"], st["h"]
                        nc.tensor.matmul(
                            st["pvp"],
                            kvn2[:, jp, :, h * P:(h + 1) * P],
                            E[:, 2 * jp:2 * jp + 2, :],
                            start=(jp == 0),
                            stop=(jp == JJ - 1),
                            perf_mode=DR,
                        )
                        nc.tensor.matmul(
                            st["dnp"],
                            ones8,
                            E[:, 2 * jp:2 * jp + 2, :],
                            start=(jp == 0),
                            stop=(jp == JJ - 1),
                            perf_mode=DR,
                        )

                    def emit_epilogue(st):
                        h, s = st["h"], st["s"]
                        # plain copy first: frees the pv PSUM bank without
                        # waiting on the reciprocal/broadcast chain
                        pvs = ep.tile([P, SW], F16, tag="pvs", name="pvs")
                        with nc.allow_low_precision(reason="x in fp16"):
                            nc.vector.tensor_copy(pvs, st["pvp"])
                        rec = ep.tile([1, SW], F32, tag="rec", name="rec")
                        nc.vector.reciprocal_approx_fast(rec, st["dnp"][0:1, :])
                        dnb = ep.tile([P, SW], F32, tag="dnb", name="dnb")
                        nc.gpsimd.partition_broadcast(dnb, rec)
                        with nc.allow_low_precision(
                            reason="x values O(0.1); fp16 keeps 5e-4 rel"
                        ):
                            nc.vector.tensor_mul(
                                xT[h][:, s * SW:(s + 1) * SW], pvs, dnb
                            )

                    def emit_phase4(s):
                        for nb in range(4 * s, 4 * s + 4):
                            pr = pjps.tile([P, C], F32, tag="pj", name="pr")
                            for kc in range(KC):
                                nc.tensor.matmul(
                                    pr,
                                    xT[kc][:, nb * P:(nb + 1) * P],
                                    Wp16[kc],
                                    start=(kc == 0),
                                    stop=(kc == KC - 1),
                                )
                            ot = osb.tile([P, C], F32, tag="ot", name="ot")
                            nc.vector.tensor_add(ot, pr, bpb)
                            nc.sync.dma_start(dOut[nb * P:(nb + 1) * P, :], ot)

                    prev = None
                    for t in range(NS * H):
                        s, h = divmod(t, H)
                        if (h, s) not in qT_done:
                            proj_chunk(qT, FT["f1"], h, s)
                            qT_done.add((h, s))
                        E = epool.tile([P, MB, SW], FP8, tag="E", name="E")
                        cur = {"E": E, "h": h, "s": s}
                        for jp in range(JJ):
                            sc = scps.tile([P, 2, SW], F32, tag="sc",
                                           name="sc")
                            for i in range(2):
                                mb = 2 * jp + i
                                nc.tensor.matmul(
                                    sc[:, i, :],
                                    kvT[h][:, mb * P:(mb + 1) * P],
                                    qT[h][:, s * SW:(s + 1) * SW],
                                    start=True,
                                    stop=True,
                                )
                            with nc.allow_low_precision(
                                reason="fp8 attention weights; rel err ~1e-2"
                            ):
                                nc.scalar.activation(
                                    E[:, 2 * jp:2 * jp + 2, :].rearrange(
                                        "p a b -> p (a b)"
                                    ),
                                    sc.rearrange("p a b -> p (a b)"),
                                    EXP,
                                    scale=float(SCALE),
                                )
                            if prev is not None:
                                emit_pv_dn(prev, jp)
                        if prev is not None:
                            emit_epilogue(prev)
                            if prev["h"] == H - 1:
                                emit_phase4(prev["s"])
                        if filler:
                            for item in filler.pop(0):
                                if item[0] == "kvT":
                                    proj_chunk(kvT, FT["f2"], item[1], item[2])
                                else:
                                    kvn_block(item[1])
                        prev = cur
                    for jp in range(JJ):
                        emit_pv_dn(prev, jp)
                    emit_epilogue(prev)
                    emit_phase4(NS - 1)

    nc.compile()
    return nc


_NC = None


def _get_nc():
    global _NC
    if _NC is None:
        _NC = build_nc()
    return _NC


def kernel(F1, F2, W_qkv, b_qkv, W_proj, b_proj, _trace=False):
    F1 = np.ascontiguousarray(np.asarray(F1, dtype=np.float32))
    F2 = np.ascontiguousarray(np.asarray(F2, dtype=np.float32))
    W = np.ascontiguousarray(np.asarray(W_qkv, dtype=np.float32))
    bq = np.ascontiguousarray(np.asarray(b_qkv, dtype=np.float32)).reshape(1, C)
    Wpj = np.ascontiguousarray(np.asarray(W_proj, dtype=np.float32))
    bp = np.ascontiguousarray(np.asarray(b_proj, dtype=np.float32)).reshape(1, C)

    nc = _get_nc()
    in_maps = [
        {"F1": F1[b], "F2": F2[b], "Wqkv": W, "bqkv": bq, "Wproj": Wpj, "bproj": bp}
        for b in range(B)
    ]
    res = run_bass_kernel_spmd(
        nc, in_maps, core_ids=list(range(B)), trace=_trace
    )
    out = np.stack([res.results[b]["OUT"] for b in range(B)], axis=0)
    if _trace:
        return out, res
    return out
